# revision 2
# baseline (speedup 1.0000x reference)
"""Trainium2 Bass kernel for the BondPoolingLayer GNN message-passing problem.

Strategy (edge-data-parallel, per the sharding hint):
  - 220000 edges are sharded across 8 NeuronCores (27500 each, padded to 27648).
  - h [200000,128] fp32 and the MLP weights are replicated to every core.
  - Each core: indirect-DMA gather of h[src]/h[dst] rows (512B/row) in big
    blocks, PE transpose to feature-major, then a fused 3-layer MLP on the
    TensorEngine with fp16 activations/weights (fp32 PSUM accumulation).
  - Math identity used: with W1 = [W1a; W1b],
      fwd = relu(h_src@W1a + h_dst@W1b + b1)
      rev = relu(h_dst@W1a + h_src@W1b + b1)
      out = relu(fwd@W2+b2)@W3 + relu(rev@W2+b2)@W3 + 2*b3
    (the two W3 matmuls accumulate into one PSUM tile).
  - Output is produced feature-major [2, E] per core and transposed on host.
"""

import os
import numpy as np

import concourse.bass as bass
import concourse.mybir as mybir
import concourse.tile as tile
from concourse import bacc
from concourse.bass import IndirectOffsetOnAxis
from concourse.bass_utils import run_bass_kernel_spmd
from concourse.masks import make_identity

N_NODES = 200000
D = 128
E_TOTAL = 220000
N_CORES = 8
E_CORE = E_TOTAL // N_CORES          # 27500

TILE_E = 128                          # edges per transpose tile
CHUNK_TILES = 4                       # tiles per MLP chunk
CHUNK_E = TILE_E * CHUNK_TILES        # 512
BLOCK_CHUNKS = 9                      # chunks per gather block
BLOCK_TILES = BLOCK_CHUNKS * CHUNK_TILES   # 36
BLOCK_E = BLOCK_TILES * TILE_E        # 4608
N_BLOCKS = 6
E_PAD = N_BLOCKS * BLOCK_E            # 27648

F32 = mybir.dt.float32
F32R = mybir.dt.float32r
F16 = mybir.dt.float16
I32 = mybir.dt.int32

# knobs
DT_ACT = F16                          # activation dtype for matmul operands
TRANSPOSE_F32R = os.environ.get("K_TRANS_F32R", "1") == "1"


def build_nc(n_blocks=N_BLOCKS):
    e_pad = n_blocks * BLOCK_E
    nc = bacc.Bacc("TRN2", target_bir_lowering=False, debug=False)

    h = nc.dram_tensor("h", [N_NODES, D], F32, kind="ExternalInput")
    sidx = nc.dram_tensor("sidx", [128, n_blocks * BLOCK_TILES], I32, kind="ExternalInput")
    didx = nc.dram_tensor("didx", [128, n_blocks * BLOCK_TILES], I32, kind="ExternalInput")
    w1a = nc.dram_tensor("w1a", [128, 128], DT_ACT, kind="ExternalInput")
    w1b = nc.dram_tensor("w1b", [128, 128], DT_ACT, kind="ExternalInput")
    w2 = nc.dram_tensor("w2", [128, 128], DT_ACT, kind="ExternalInput")
    w3 = nc.dram_tensor("w3", [128, 2], DT_ACT, kind="ExternalInput")
    b1 = nc.dram_tensor("b1", [128, 1], F32, kind="ExternalInput")
    b2 = nc.dram_tensor("b2", [128, 1], F32, kind="ExternalInput")
    b3x2 = nc.dram_tensor("b3x2", [2, 1], F32, kind="ExternalInput")
    out = nc.dram_tensor("out", [2, e_pad], F32, kind="ExternalOutput")

    with tile.TileContext(nc) as tc:
        _program(tc, nc, n_blocks, h, sidx, didx, w1a, w1b, w2, w3, b1, b2, b3x2, out)
    nc.compile()
    return nc


def _program(tc, nc, n_blocks, h, sidx, didx, w1a, w1b, w2, w3, b1, b2, b3x2, out):
    Relu = mybir.ActivationFunctionType.Relu

    with (
        tc.tile_pool(name="const", bufs=1) as const_pool,
        tc.tile_pool(name="blk", bufs=2) as blk_pool,
        tc.tile_pool(name="act", bufs=3) as act_pool,
        tc.tile_pool(name="outp", bufs=4) as out_pool,
        tc.tile_pool(name="trp", bufs=2, space="PSUM") as tr_pool,
        tc.tile_pool(name="l1p", bufs=1, space="PSUM") as l1_pool,
        tc.tile_pool(name="l2p", bufs=1, space="PSUM") as l2_pool,
    ):
        # ---- constants ----
        ident = const_pool.tile([128, 128], F32)
        make_identity(nc, ident[:])

        w1a_t = const_pool.tile([128, 128], DT_ACT)
        nc.sync.dma_start(w1a_t[:], w1a.ap())
        w1b_t = const_pool.tile([128, 128], DT_ACT)
        nc.sync.dma_start(w1b_t[:], w1b.ap())
        w2_t = const_pool.tile([128, 128], DT_ACT)
        nc.sync.dma_start(w2_t[:], w2.ap())
        w3_t = const_pool.tile([128, 2], DT_ACT)
        nc.sync.dma_start(w3_t[:], w3.ap())
        b1_t = const_pool.tile([128, 1], F32)
        nc.sync.dma_start(b1_t[:], b1.ap())
        b2_t = const_pool.tile([128, 1], F32)
        nc.sync.dma_start(b2_t[:], b2.ap())
        b3_t = const_pool.tile([2, 1], F32)
        nc.sync.dma_start(b3_t[:], b3x2.ap())

        sidx_t = const_pool.tile([128, n_blocks * BLOCK_TILES], I32)
        nc.sync.dma_start(sidx_t[:], sidx.ap())
        didx_t = const_pool.tile([128, n_blocks * BLOCK_TILES], I32)
        nc.sync.dma_start(didx_t[:], didx.ap())

        out_ap = out.ap()

        for b in range(n_blocks):
            # ---- gather h rows for this block's edges (src and dst) ----
            # HW contract: one index per partition per indirect DMA -> 128
            # rows per call (idx [128,1], out [128, D]).
            xs = blk_pool.tile([128, BLOCK_TILES, D], F32, tag="xs")
            xd = blk_pool.tile([128, BLOCK_TILES, D], F32, tag="xd")
            for j in range(BLOCK_TILES):
                col = b * BLOCK_TILES + j
                nc.gpsimd.indirect_dma_start(
                    out=xs[:, j, :],
                    out_offset=None,
                    in_=h.ap(),
                    in_offset=IndirectOffsetOnAxis(
                        ap=sidx_t[:, col:col + 1], axis=0),
                )
                nc.gpsimd.indirect_dma_start(
                    out=xd[:, j, :],
                    out_offset=None,
                    in_=h.ap(),
                    in_offset=IndirectOffsetOnAxis(
                        ap=didx_t[:, col:col + 1], axis=0),
                )

            for c in range(BLOCK_CHUNKS):
                # ---- transpose 4+4 tiles to feature-major ----
                trp = tr_pool.tile([128, 2 * CHUNK_E], F32, tag="trp", space="PSUM")
                for t in range(CHUNK_TILES):
                    j = c * CHUNK_TILES + t
                    src_in = xs[:, j, :]
                    dst_in = xd[:, j, :]
                    o_s = trp[:, t * TILE_E:(t + 1) * TILE_E]
                    o_d = trp[:, CHUNK_E + t * TILE_E:CHUNK_E + (t + 1) * TILE_E]
                    if TRANSPOSE_F32R:
                        nc.tensor.transpose(o_s.bitcast(F32R), src_in.bitcast(F32R),
                                            ident[:].bitcast(F32R))
                        nc.tensor.transpose(o_d.bitcast(F32R), dst_in.bitcast(F32R),
                                            ident[:].bitcast(F32R))
                    else:
                        nc.tensor.transpose(o_s, src_in, ident[:])
                        nc.tensor.transpose(o_d, dst_in, ident[:])

                # ---- PSUM -> SBUF (cast to DT_ACT) ----
                xt = act_pool.tile([128, 2 * CHUNK_E], DT_ACT, tag="xt")
                nc.vector.tensor_copy(xt[:], trp[:])
                xsT = xt[:, 0:CHUNK_E]
                xdT = xt[:, CHUNK_E:2 * CHUNK_E]

                # ---- layer 1 (fwd | rev) ----
                l1 = l1_pool.tile([128, 2 * CHUNK_E], F32, tag="l1", space="PSUM")
                nc.tensor.matmul(l1[:, 0:CHUNK_E], w1a_t[:], xsT, start=True, stop=False)
                nc.tensor.matmul(l1[:, 0:CHUNK_E], w1b_t[:], xdT, start=False, stop=True)
                nc.tensor.matmul(l1[:, CHUNK_E:], w1a_t[:], xdT, start=True, stop=False)
                nc.tensor.matmul(l1[:, CHUNK_E:], w1b_t[:], xsT, start=False, stop=True)

                h1 = act_pool.tile([128, 2 * CHUNK_E], DT_ACT, tag="h1")
                nc.scalar.activation(h1[:], l1[:], Relu, bias=b1_t[:, 0:1])

                # ---- layer 2 ----
                l2 = l2_pool.tile([128, 2 * CHUNK_E], F32, tag="l2", space="PSUM")
                nc.tensor.matmul(l2[:, 0:CHUNK_E], w2_t[:], h1[:, 0:CHUNK_E],
                                 start=True, stop=True)
                nc.tensor.matmul(l2[:, CHUNK_E:], w2_t[:], h1[:, CHUNK_E:],
                                 start=True, stop=True)

                h2 = act_pool.tile([128, 2 * CHUNK_E], DT_ACT, tag="h2")
                nc.scalar.activation(h2[:], l2[:], Relu, bias=b2_t[:, 0:1])

                # ---- layer 3: accumulate fwd+rev into one [2, 512] psum ----
                l3 = l2_pool.tile([2, CHUNK_E], F32, tag="l2", space="PSUM")
                nc.tensor.matmul(l3[:], w3_t[:], h2[:, 0:CHUNK_E], start=True, stop=False)
                nc.tensor.matmul(l3[:], w3_t[:], h2[:, CHUNK_E:], start=False, stop=True)

                o = out_pool.tile([2, CHUNK_E], F32, tag="o")
                nc.vector.tensor_scalar_add(o[:], l3[:], b3_t[:, 0:1])

                col = b * BLOCK_E + c * CHUNK_E
                nc.sync.dma_start(out_ap[:, col:col + CHUNK_E], o[:])


_NC_CACHE = {}


def _get_nc(n_blocks=N_BLOCKS):
    if n_blocks not in _NC_CACHE:
        _NC_CACHE[n_blocks] = build_nc(n_blocks)
    return _NC_CACHE[n_blocks]


def _np_dt(dt):
    return np.float16 if dt == F16 else np.dtype(mybir.dt.np(dt))


def _make_idx_tile(idx_pad: np.ndarray, n_blocks: int) -> np.ndarray:
    """[e_pad] int -> [128, n_blocks*BLOCK_TILES] laid out so that
    tile j of block b, partition p == edge b*BLOCK_E + j*TILE_E + p."""
    cols = []
    for b in range(n_blocks):
        blk = idx_pad[b * BLOCK_E:(b + 1) * BLOCK_E].reshape(BLOCK_TILES, TILE_E).T
        cols.append(blk)
    return np.ascontiguousarray(np.concatenate(cols, axis=1), dtype=np.int32)


def make_in_map(h, src_shard, dst_shard, W1, b1, W2, b2, W3, b3, n_blocks=N_BLOCKS):
    e_pad = n_blocks * BLOCK_E
    spad = np.zeros(e_pad, dtype=np.int64)
    dpad = np.zeros(e_pad, dtype=np.int64)
    spad[:len(src_shard)] = src_shard
    dpad[:len(dst_shard)] = dst_shard
    act = _np_dt(DT_ACT)
    return {
        "h": np.ascontiguousarray(h, dtype=np.float32),
        "sidx": _make_idx_tile(spad, n_blocks),
        "didx": _make_idx_tile(dpad, n_blocks),
        "w1a": np.ascontiguousarray(W1[:128], dtype=act),
        "w1b": np.ascontiguousarray(W1[128:], dtype=act),
        "w2": np.ascontiguousarray(W2, dtype=act),
        "w3": np.ascontiguousarray(W3, dtype=act),
        "b1": np.ascontiguousarray(b1.reshape(128, 1), dtype=np.float32),
        "b2": np.ascontiguousarray(b2.reshape(128, 1), dtype=np.float32),
        "b3x2": np.ascontiguousarray((2.0 * b3).reshape(2, 1), dtype=np.float32),
    }


def kernel(h, src, dst, W1, b1, W2, b2, W3, b3, **run_kwargs):
    h = np.asarray(h, dtype=np.float32)
    src = np.asarray(src).astype(np.int64)
    dst = np.asarray(dst).astype(np.int64)
    W1 = np.asarray(W1); W2 = np.asarray(W2); W3 = np.asarray(W3)
    b1 = np.asarray(b1); b2 = np.asarray(b2); b3 = np.asarray(b3)

    nc = _get_nc()
    in_maps = []
    for c in range(N_CORES):
        sl = slice(c * E_CORE, (c + 1) * E_CORE)
        in_maps.append(make_in_map(h, src[sl], dst[sl], W1, b1, W2, b2, W3, b3))

    res = run_bass_kernel_spmd(nc, in_maps, core_ids=list(range(N_CORES)), **run_kwargs)

    out = np.empty((E_TOTAL, 2), dtype=np.float32)
    for c in range(N_CORES):
        o = res.results[c]["out"]          # [2, E_PAD]
        out[c * E_CORE:(c + 1) * E_CORE] = o.T[:E_CORE]
    if run_kwargs:
        kernel.last_results = res
    return out


# revision 5
# speedup vs baseline: 6.2504x; 6.2504x over previous
"""Trainium2 Bass kernel for the BondPoolingLayer GNN message-passing problem.

Strategy (edge-data-parallel, per the sharding hint):
  - 220000 edges are sharded across 8 NeuronCores (27500 each, padded to 27648).
  - h [200000,128] fp32 and the MLP weights are replicated to every core.
  - Each core: indirect-DMA gather of h[src]/h[dst] rows (512B/row) in big
    blocks, PE transpose to feature-major, then a fused 3-layer MLP on the
    TensorEngine with fp16 activations/weights (fp32 PSUM accumulation).
  - Math identity used: with W1 = [W1a; W1b],
      fwd = relu(h_src@W1a + h_dst@W1b + b1)
      rev = relu(h_dst@W1a + h_src@W1b + b1)
      out = relu(fwd@W2+b2)@W3 + relu(rev@W2+b2)@W3 + 2*b3
    (the two W3 matmuls accumulate into one PSUM tile).
  - Output is produced feature-major [2, E] per core and transposed on host.
"""

import os
import numpy as np

import concourse.bass as bass
import concourse.mybir as mybir
import concourse.tile as tile
from concourse import bacc
from concourse.bass import IndirectOffsetOnAxis
from concourse.bass_utils import run_bass_kernel_spmd
from concourse.masks import make_identity

N_NODES = 200000
D = 128
E_TOTAL = 220000
N_CORES = 8
E_CORE = E_TOTAL // N_CORES          # 27500

TILE_E = 128                          # edges per transpose tile
CHUNK_TILES = 4                       # tiles per MLP chunk
CHUNK_E = TILE_E * CHUNK_TILES        # 512
BLOCK_CHUNKS = 9                      # chunks per gather block
BLOCK_TILES = BLOCK_CHUNKS * CHUNK_TILES   # 36
BLOCK_E = BLOCK_TILES * TILE_E        # 4608
N_BLOCKS = 6
E_PAD = N_BLOCKS * BLOCK_E            # 27648

F32 = mybir.dt.float32
F32R = mybir.dt.float32r
F16 = mybir.dt.float16
I32 = mybir.dt.int32

# knobs
_DT = os.environ.get("K_DT", "f32")
DT_ACT = {"f32": F32, "f16": F16, "bf16": mybir.dt.bfloat16}[_DT]
TRANSPOSE_F32R = os.environ.get("K_TRANS_F32R", "0") == "1"
NOGATHER = os.environ.get("K_NOGATHER", "0") == "1"   # perf probe: plain DMA


def build_nc(n_blocks=N_BLOCKS):
    e_pad = n_blocks * BLOCK_E
    nc = bacc.Bacc("TRN2", target_bir_lowering=False, debug=False)

    h = nc.dram_tensor("h", [N_NODES, D], F32, kind="ExternalInput")
    sidx = nc.dram_tensor("sidx", [128, n_blocks * BLOCK_TILES], I32, kind="ExternalInput")
    didx = nc.dram_tensor("didx", [128, n_blocks * BLOCK_TILES], I32, kind="ExternalInput")
    w1a = nc.dram_tensor("w1a", [128, 128], DT_ACT, kind="ExternalInput")
    w1b = nc.dram_tensor("w1b", [128, 128], DT_ACT, kind="ExternalInput")
    w2 = nc.dram_tensor("w2", [128, 128], DT_ACT, kind="ExternalInput")
    w3 = nc.dram_tensor("w3", [128, 2], DT_ACT, kind="ExternalInput")
    b1 = nc.dram_tensor("b1", [128, 1], F32, kind="ExternalInput")
    b2 = nc.dram_tensor("b2", [128, 1], F32, kind="ExternalInput")
    b3x2 = nc.dram_tensor("b3x2", [2, 1], F32, kind="ExternalInput")
    out = nc.dram_tensor("out", [2, e_pad], F32, kind="ExternalOutput")

    with tile.TileContext(nc) as tc:
        _program(tc, nc, n_blocks, h, sidx, didx, w1a, w1b, w2, w3, b1, b2, b3x2, out)
    nc.compile()
    return nc


def _program(tc, nc, n_blocks, h, sidx, didx, w1a, w1b, w2, w3, b1, b2, b3x2, out):
    Relu = mybir.ActivationFunctionType.Relu

    with (
        tc.tile_pool(name="const", bufs=1) as const_pool,
        tc.tile_pool(name="blk", bufs=2) as blk_pool,
        tc.tile_pool(name="act", bufs=3) as act_pool,
        tc.tile_pool(name="outp", bufs=4) as out_pool,
        tc.tile_pool(name="trp", bufs=2, space="PSUM") as tr_pool,
        tc.tile_pool(name="l1p", bufs=1, space="PSUM") as l1_pool,
        tc.tile_pool(name="l2p", bufs=1, space="PSUM") as l2_pool,
    ):
        # ---- constants ----
        ident = const_pool.tile([128, 128], F32)
        make_identity(nc, ident[:])

        w1a_t = const_pool.tile([128, 128], DT_ACT)
        nc.sync.dma_start(w1a_t[:], w1a.ap())
        w1b_t = const_pool.tile([128, 128], DT_ACT)
        nc.sync.dma_start(w1b_t[:], w1b.ap())
        w2_t = const_pool.tile([128, 128], DT_ACT)
        nc.sync.dma_start(w2_t[:], w2.ap())
        w3_t = const_pool.tile([128, 2], DT_ACT)
        nc.sync.dma_start(w3_t[:], w3.ap())
        b1_t = const_pool.tile([128, 1], F32)
        nc.sync.dma_start(b1_t[:], b1.ap())
        b2_t = const_pool.tile([128, 1], F32)
        nc.sync.dma_start(b2_t[:], b2.ap())
        b3_t = const_pool.tile([2, 1], F32)
        nc.sync.dma_start(b3_t[:], b3x2.ap())

        sidx_t = const_pool.tile([128, n_blocks * BLOCK_TILES], I32)
        nc.sync.dma_start(sidx_t[:], sidx.ap())
        didx_t = const_pool.tile([128, n_blocks * BLOCK_TILES], I32)
        nc.sync.dma_start(didx_t[:], didx.ap())

        out_ap = out.ap()

        for b in range(n_blocks):
            # ---- gather h rows for this block's edges (src and dst) ----
            # HW contract: one index per partition per indirect DMA -> 128
            # rows per call (idx [128,1], out [128, D]).
            xs = blk_pool.tile([128, BLOCK_TILES, D], F32, tag="xs")
            xd = blk_pool.tile([128, BLOCK_TILES, D], F32, tag="xd")
            if NOGATHER:
                # perf probe only (wrong values): same bytes via plain DMA
                base = (b * BLOCK_E) % (N_NODES - BLOCK_E)
                hs = h.ap()[base:base + BLOCK_E, :].rearrange(
                    "(k p) d -> p k d", p=128)
                nc.sync.dma_start(xs[:], hs)
                nc.sync.dma_start(xd[:], hs)
            else:
                for j in range(BLOCK_TILES):
                    col = b * BLOCK_TILES + j
                    nc.gpsimd.indirect_dma_start(
                        out=xs[:, j, :],
                        out_offset=None,
                        in_=h.ap(),
                        in_offset=IndirectOffsetOnAxis(
                            ap=sidx_t[:, col:col + 1], axis=0),
                    )
                    nc.gpsimd.indirect_dma_start(
                        out=xd[:, j, :],
                        out_offset=None,
                        in_=h.ap(),
                        in_offset=IndirectOffsetOnAxis(
                            ap=didx_t[:, col:col + 1], axis=0),
                    )

            for c in range(BLOCK_CHUNKS):
                # ---- transpose 4+4 tiles to feature-major ----
                trp = tr_pool.tile([128, 2 * CHUNK_E], F32, tag="trp", space="PSUM")
                for t in range(CHUNK_TILES):
                    j = c * CHUNK_TILES + t
                    src_in = xs[:, j, :]
                    dst_in = xd[:, j, :]
                    o_s = trp[:, t * TILE_E:(t + 1) * TILE_E]
                    o_d = trp[:, CHUNK_E + t * TILE_E:CHUNK_E + (t + 1) * TILE_E]
                    if TRANSPOSE_F32R:
                        nc.tensor.transpose(o_s.bitcast(F32R), src_in.bitcast(F32R),
                                            ident[:].bitcast(F32R))
                        nc.tensor.transpose(o_d.bitcast(F32R), dst_in.bitcast(F32R),
                                            ident[:].bitcast(F32R))
                    else:
                        nc.tensor.transpose(o_s, src_in, ident[:])
                        nc.tensor.transpose(o_d, dst_in, ident[:])

                # ---- PSUM -> SBUF (cast to DT_ACT) ----
                xt = act_pool.tile([128, 2 * CHUNK_E], DT_ACT, tag="xt")
                nc.vector.tensor_copy(xt[:], trp[:])
                xsT = xt[:, 0:CHUNK_E]
                xdT = xt[:, CHUNK_E:2 * CHUNK_E]

                # ---- layer 1 (fwd | rev) ----
                l1 = l1_pool.tile([128, 2 * CHUNK_E], F32, tag="l1", space="PSUM")
                nc.tensor.matmul(l1[:, 0:CHUNK_E], w1a_t[:], xsT, start=True, stop=False)
                nc.tensor.matmul(l1[:, 0:CHUNK_E], w1b_t[:], xdT, start=False, stop=True)
                nc.tensor.matmul(l1[:, CHUNK_E:], w1a_t[:], xdT, start=True, stop=False)
                nc.tensor.matmul(l1[:, CHUNK_E:], w1b_t[:], xsT, start=False, stop=True)

                h1 = act_pool.tile([128, 2 * CHUNK_E], DT_ACT, tag="h1")
                nc.scalar.activation(h1[:], l1[:], Relu, bias=b1_t[:, 0:1])

                # ---- layer 2 ----
                l2 = l2_pool.tile([128, 2 * CHUNK_E], F32, tag="l2", space="PSUM")
                nc.tensor.matmul(l2[:, 0:CHUNK_E], w2_t[:], h1[:, 0:CHUNK_E],
                                 start=True, stop=True)
                nc.tensor.matmul(l2[:, CHUNK_E:], w2_t[:], h1[:, CHUNK_E:],
                                 start=True, stop=True)

                h2 = act_pool.tile([128, 2 * CHUNK_E], DT_ACT, tag="h2")
                nc.scalar.activation(h2[:], l2[:], Relu, bias=b2_t[:, 0:1])

                # ---- layer 3: accumulate fwd+rev into one [2, 512] psum ----
                l3 = l2_pool.tile([2, CHUNK_E], F32, tag="l2", space="PSUM")
                nc.tensor.matmul(l3[:], w3_t[:], h2[:, 0:CHUNK_E], start=True, stop=False)
                nc.tensor.matmul(l3[:], w3_t[:], h2[:, CHUNK_E:], start=False, stop=True)

                o = out_pool.tile([2, CHUNK_E], F32, tag="o")
                nc.vector.tensor_scalar_add(o[:], l3[:], b3_t[:, 0:1])

                col = b * BLOCK_E + c * CHUNK_E
                nc.sync.dma_start(out_ap[:, col:col + CHUNK_E], o[:])


_NC_CACHE = {}


def _get_nc(n_blocks=N_BLOCKS):
    if n_blocks not in _NC_CACHE:
        _NC_CACHE[n_blocks] = build_nc(n_blocks)
    return _NC_CACHE[n_blocks]


def _np_dt(dt):
    return np.float16 if dt == F16 else np.dtype(mybir.dt.np(dt))


def _make_idx_tile(idx_pad: np.ndarray, n_blocks: int) -> np.ndarray:
    """[e_pad] int -> [128, n_blocks*BLOCK_TILES] laid out so that
    tile j of block b, partition p == edge b*BLOCK_E + j*TILE_E + p."""
    cols = []
    for b in range(n_blocks):
        blk = idx_pad[b * BLOCK_E:(b + 1) * BLOCK_E].reshape(BLOCK_TILES, TILE_E).T
        cols.append(blk)
    return np.ascontiguousarray(np.concatenate(cols, axis=1), dtype=np.int32)


def make_in_map(h, src_shard, dst_shard, W1, b1, W2, b2, W3, b3, n_blocks=N_BLOCKS):
    e_pad = n_blocks * BLOCK_E
    spad = np.zeros(e_pad, dtype=np.int64)
    dpad = np.zeros(e_pad, dtype=np.int64)
    spad[:len(src_shard)] = src_shard
    dpad[:len(dst_shard)] = dst_shard
    act = _np_dt(DT_ACT)
    return {
        "h": np.ascontiguousarray(h, dtype=np.float32),
        "sidx": _make_idx_tile(spad, n_blocks),
        "didx": _make_idx_tile(dpad, n_blocks),
        "w1a": np.ascontiguousarray(W1[:128], dtype=act),
        "w1b": np.ascontiguousarray(W1[128:], dtype=act),
        "w2": np.ascontiguousarray(W2, dtype=act),
        "w3": np.ascontiguousarray(W3, dtype=act),
        "b1": np.ascontiguousarray(b1.reshape(128, 1), dtype=np.float32),
        "b2": np.ascontiguousarray(b2.reshape(128, 1), dtype=np.float32),
        "b3x2": np.ascontiguousarray((2.0 * b3).reshape(2, 1), dtype=np.float32),
    }


def kernel(h, src, dst, W1, b1, W2, b2, W3, b3, **run_kwargs):
    h = np.asarray(h, dtype=np.float32)
    src = np.asarray(src).astype(np.int64)
    dst = np.asarray(dst).astype(np.int64)
    W1 = np.asarray(W1); W2 = np.asarray(W2); W3 = np.asarray(W3)
    b1 = np.asarray(b1); b2 = np.asarray(b2); b3 = np.asarray(b3)

    nc = _get_nc()
    in_maps = []
    for c in range(N_CORES):
        sl = slice(c * E_CORE, (c + 1) * E_CORE)
        in_maps.append(make_in_map(h, src[sl], dst[sl], W1, b1, W2, b2, W3, b3))

    res = run_bass_kernel_spmd(nc, in_maps, core_ids=list(range(N_CORES)), **run_kwargs)

    out = np.empty((E_TOTAL, 2), dtype=np.float32)
    for c in range(N_CORES):
        o = res.results[c]["out"]          # [2, E_PAD]
        out[c * E_CORE:(c + 1) * E_CORE] = o.T[:E_CORE]
    if run_kwargs:
        kernel.last_results = res
    return out
